# revision 3
# baseline (speedup 1.0000x reference)
"""Trainium2 Bass kernel for nn_BidirectionalLoss (topk_masking).

Math restructuring (t is binary 0/1, p in (eps, 1-eps)):
  * u = p - t
      - BCE elementwise loss: -(t*log(p) + (1-t)*log(1-p)) = -ln(1 - |u|)
        (t=0 -> |u|=p, ln(1-p); t=1 -> |u|=1-p, ln(p))
      - top-k negatives: for t=1, u=p-1 < 0 < p = u for t=0, so max8(u)
        yields the highest-scoring negatives directly.
  * hard-negative mask (k=2, top_k=6): the selected negatives are the top-2
    negatives w0 >= w1 gated by membership in the overall top-6, i.e.
    w_j >= v6 where v6 = 6th largest score (from max8(p)).
  * per-row stats (bce row-sum, selected-negative ln-sum, mask count) are
    DMA'd out; the final scalar reduction over rows is done on host in f64.

Sharding: pure data parallel over the batch dim, 512 rows per core x 8 cores.
"""

import sys

for _p in ("/opt/trn_rl_repo", "/root/.axon_site/_ro/trn_rl_repo"):
    if _p not in sys.path:
        sys.path.append(_p)

import numpy as np

from concourse import bass, mybir
from concourse.tile import TileContext
from concourse.bass_utils import run_bass_kernel_spmd

B, C = 4096, 8192
N_CORES = 8
R = B // N_CORES            # rows per core
P = 128                     # partitions per row-tile
N_RT = R // P               # row-tiles per core
CH = 4096                   # column chunk
N_CH = C // CH
f32 = mybir.dt.float32
AF = mybir.ActivationFunctionType
ALU = mybir.AluOpType

_CACHE = {}


def _split_waits(nc, max_waits=1):
    """The TPB_CTRL-class instructions only support one sync-wait slot in
    walrus codegen; split any instruction carrying more waits into a chain
    of single-wait NoOps in front of it."""
    n = 0
    for f in nc.m.functions:
        for blk in f.blocks:
            il = blk.instructions
            i = 0
            while i < len(il):
                inst = il[i]
                si = getattr(inst, "sync_info", None)
                if si is not None and si.on_wait and len(si.on_wait) > max_waits:
                    waits = list(si.on_wait)
                    head, tail = waits[:-max_waits], waits[-max_waits:]
                    while head:
                        chunk, head = head[:max_waits], head[max_waits:]
                        noop = mybir.InstNoOp(
                            name=f"wait_split_{n}",
                            sync_info=mybir.SyncInfo(on_wait=chunk, on_update=[]),
                            bass_nofuse=True,
                        )
                        n += 1
                        noop.engine = inst.engine
                        il.insert(i, noop)
                        i += 1
                    inst.sync_info = mybir.SyncInfo(
                        on_wait=tail, on_update=list(si.on_update)
                    )
                i += 1
    return n


def _build():
    nc = bass.Bass("TRN2", target_bir_lowering=False, debug=False,
                   num_devices=N_CORES)
    ins = {
        name: nc.dram_tensor(name, [R, C], f32, kind="ExternalInput")
        for name in ("tk_s", "tk_t", "g_s", "g_t")
    }
    stats = nc.dram_tensor("stats", [R, 8], f32, kind="ExternalOutput")

    dirs = [(ins["tk_s"], ins["tk_t"]), (ins["g_s"], ins["g_t"])]

    with TileContext(nc) as tc:
        with (
            tc.tile_pool(name="big", bufs=3) as big,
            tc.tile_pool(name="aux", bufs=2) as aux,
            tc.tile_pool(name="small", bufs=4) as small,
        ):
            for d, (s_d, t_d) in enumerate(dirs):
                for rt in range(N_RT):
                    rows = slice(rt * P, (rt + 1) * P)
                    v16 = small.tile([P, 8 * N_CH], f32, tag="v16")
                    w16 = small.tile([P, 8 * N_CH], f32, tag="w16")
                    accs = small.tile([P, N_CH], f32, tag="accs")
                    for ch in range(N_CH):
                        cols = slice(ch * CH, (ch + 1) * CH)
                        p = big.tile([P, CH], f32, tag="p")
                        t = big.tile([P, CH], f32, tag="t")
                        u = big.tile([P, CH], f32, tag="u")
                        a = aux.tile([P, CH], f32, tag="a")
                        nc.sync.dma_start(out=p, in_=s_d[rows, cols])
                        nc.sync.dma_start(out=t, in_=t_d[rows, cols])
                        # u = p - t on GPSIMD (keeps DVE under the DMA roofline)
                        nc.gpsimd.tensor_tensor(out=u, in0=p, in1=t,
                                                op=ALU.subtract)
                        # chunk top-8 of scores and of negatives
                        nc.vector.max(out=v16[:, 8 * ch:8 * ch + 8], in_=p)
                        nc.vector.max(out=w16[:, 8 * ch:8 * ch + 8], in_=u)
                        # BCE row-sum: sum ln(1 - |u|)
                        nc.scalar.activation(out=a, in_=u, func=AF.Abs)
                        nc.scalar.activation(
                            out=u, in_=a, func=AF.Ln, scale=-1.0, bias=1.0,
                            accum_out=accs[:, ch:ch + 1],
                        )
                    # epilogue: merge chunk top-8s, select first<=2 negatives
                    w8 = small.tile([P, 8], f32, tag="w8")
                    v8 = small.tile([P, 8], f32, tag="v8")
                    nc.vector.max(out=w8, in_=w16)
                    nc.vector.max(out=v8, in_=v16)
                    ge2 = small.tile([P, 2], f32, tag="ge2")
                    nc.vector.tensor_scalar(
                        ge2, w8[:, 0:2], v8[:, 5:6], None, op0=ALU.is_ge)
                    lnw = small.tile([P, 2], f32, tag="lnw")
                    nc.scalar.activation(out=lnw, in_=w8[:, 0:2], func=AF.Ln,
                                         scale=-1.0, bias=1.0)
                    ot = small.tile([P, 4], f32, tag="ot")
                    tmp = small.tile([P, 2], f32, tag="tmp")
                    # bce row-sum = accs[:,0] + ... + accs[:,N_CH-1]
                    nc.vector.tensor_reduce(
                        ot[:, 0:1], accs, axis=mybir.AxisListType.X, op=ALU.add)
                    # selected-negative ln-sum = sum(ge2 * lnw)
                    nc.vector.scalar_tensor_tensor(
                        out=tmp, in0=ge2, scalar=1.0, in1=lnw,
                        op0=ALU.mult, op1=ALU.mult, accum_out=ot[:, 1:2])
                    # mask count = sum(ge2)
                    nc.vector.tensor_reduce(
                        ot[:, 2:3], ge2, axis=mybir.AxisListType.X, op=ALU.add)
                    nc.sync.dma_start(
                        out=stats[rows, 4 * d:4 * d + 3], in_=ot[:, 0:3])

    _split_waits(nc)
    return nc


def _get_nc():
    if "nc" not in _CACHE:
        _CACHE["nc"] = _build()
    return _CACHE["nc"]


def kernel(tk_scores, g_scores, tk_targets, g_targets, confidences):
    nc = _get_nc()
    tk_scores = np.asarray(tk_scores)
    g_scores = np.asarray(g_scores)
    tk_targets = np.asarray(tk_targets)
    g_targets = np.asarray(g_targets)

    in_maps = [
        {
            "tk_s": tk_scores[c * R:(c + 1) * R],
            "tk_t": tk_targets[c * R:(c + 1) * R],
            "g_s": g_scores[c * R:(c + 1) * R],
            "g_t": g_targets[c * R:(c + 1) * R],
        }
        for c in range(N_CORES)
    ]
    res = run_bass_kernel_spmd(nc, in_maps, list(range(N_CORES)))
    stats = np.concatenate(
        [res.results[c]["stats"] for c in range(N_CORES)], axis=0
    ).astype(np.float64)

    conf = np.asarray(confidences, dtype=np.float64)

    def finish(off):
        acc = stats[:, off + 0]      # sum ln(q) per row  (= -row BCE sum)
        negs = stats[:, off + 1]     # sum sel*ln(1-w)    (= -selected loss)
        ms = stats[:, off + 2]
        pos = (conf * -acc).sum() / (B * C)
        neg = (-negs).sum() / (ms.sum() + 1e-8)
        return pos + 0.5 * neg

    tk = finish(0)
    g = finish(4)
    total = 0.6 * tk + 0.4 * g
    return (np.float32(total), np.float32(tk), np.float32(g))


# revision 6
# speedup vs baseline: 1.0817x; 1.0817x over previous
"""Trainium2 Bass kernel for nn_BidirectionalLoss (topk_masking).

Math restructuring (t is binary 0/1, p in (eps, 1-eps)):
  * u = p - t
      - BCE elementwise loss: -(t*log(p) + (1-t)*log(1-p)) = -ln(1 - |u|)
        (t=0 -> |u|=p, ln(1-p); t=1 -> |u|=1-p, ln(p))
      - top-k negatives: for t=1, u=p-1 < 0 < p = u for t=0, so max8(u)
        yields the highest-scoring negatives directly.
  * hard-negative mask (k=2, top_k=6): the selected negatives are the top-2
    negatives w0 >= w1 gated by membership in the overall top-6, i.e.
    w_j >= v6 where v6 = 6th largest score (from max8(p)).
  * per-row stats (bce row-sum, selected-negative ln-sum, mask count) are
    DMA'd out; the final scalar reduction over rows is done on host in f64.

Sharding: pure data parallel over the batch dim, 512 rows per core x 8 cores.
"""

import sys

for _p in ("/opt/trn_rl_repo", "/root/.axon_site/_ro/trn_rl_repo"):
    if _p not in sys.path:
        sys.path.append(_p)

import numpy as np

from concourse import bass, mybir
from concourse.tile import TileContext
from concourse.bass_utils import run_bass_kernel_spmd

B, C = 4096, 8192
N_CORES = 8
R = B // N_CORES            # rows per core
P = 128                     # partitions per row-tile
N_RT = R // P               # row-tiles per core
CH = 2048                   # column chunk
N_CH = C // CH
f32 = mybir.dt.float32
AF = mybir.ActivationFunctionType
ALU = mybir.AluOpType

_CACHE = {}


def _split_waits(nc, max_waits=1):
    """The TPB_CTRL-class instructions only support one sync-wait slot in
    walrus codegen; split any instruction carrying more waits into a chain
    of single-wait NoOps in front of it."""
    n = 0
    for f in nc.m.functions:
        for blk in f.blocks:
            il = blk.instructions
            i = 0
            while i < len(il):
                inst = il[i]
                si = getattr(inst, "sync_info", None)
                if si is not None and si.on_wait and len(si.on_wait) > max_waits:
                    waits = list(si.on_wait)
                    head, tail = waits[:-max_waits], waits[-max_waits:]
                    while head:
                        chunk, head = head[:max_waits], head[max_waits:]
                        noop = mybir.InstNoOp(
                            name=f"wait_split_{n}",
                            sync_info=mybir.SyncInfo(on_wait=chunk, on_update=[]),
                            bass_nofuse=True,
                        )
                        n += 1
                        noop.engine = inst.engine
                        il.insert(i, noop)
                        i += 1
                    inst.sync_info = mybir.SyncInfo(
                        on_wait=tail, on_update=list(si.on_update)
                    )
                i += 1
    return n


def _build():
    nc = bass.Bass("TRN2", target_bir_lowering=False, debug=False,
                   num_devices=N_CORES)
    ins = {
        name: nc.dram_tensor(name, [R, C], f32, kind="ExternalInput")
        for name in ("tk_s", "tk_t", "g_s", "g_t")
    }
    stats = nc.dram_tensor("stats", [R, 8], f32, kind="ExternalOutput")

    dirs = [(ins["tk_s"], ins["tk_t"]), (ins["g_s"], ins["g_t"])]

    with TileContext(nc) as tc:
        with (
            tc.tile_pool(name="big", bufs=6) as big,
            tc.tile_pool(name="aux", bufs=3) as aux,
            tc.tile_pool(name="small", bufs=4) as small,
        ):
            for d, (s_d, t_d) in enumerate(dirs):
                for rt in range(N_RT):
                    rows = slice(rt * P, (rt + 1) * P)
                    v16 = small.tile([P, 8 * N_CH], f32, tag="v16")
                    w16 = small.tile([P, 8 * N_CH], f32, tag="w16")
                    accs = small.tile([P, N_CH], f32, tag="accs")
                    for ch in range(N_CH):
                        cols = slice(ch * CH, (ch + 1) * CH)
                        p = big.tile([P, CH], f32, tag="p")
                        t = big.tile([P, CH], f32, tag="t")
                        u = big.tile([P, CH], f32, tag="u")
                        a = aux.tile([P, CH], f32, tag="a")
                        nc.sync.dma_start(out=p, in_=s_d[rows, cols])
                        nc.sync.dma_start(out=t, in_=t_d[rows, cols])
                        # u = p - t on GPSIMD (keeps DVE under the DMA roofline)
                        nc.gpsimd.tensor_tensor(out=u, in0=p, in1=t,
                                                op=ALU.subtract)
                        # chunk top-8 of scores and of negatives
                        nc.vector.max(out=v16[:, 8 * ch:8 * ch + 8], in_=p)
                        nc.vector.max(out=w16[:, 8 * ch:8 * ch + 8], in_=u)
                        # BCE row-sum: sum ln(1 - |u|)
                        nc.scalar.activation(out=a, in_=u, func=AF.Abs)
                        nc.scalar.activation(
                            out=u, in_=a, func=AF.Ln, scale=-1.0, bias=1.0,
                            accum_out=accs[:, ch:ch + 1],
                        )
                    # epilogue: merge chunk top-8s, select first<=2 negatives
                    w8 = small.tile([P, 8], f32, tag="w8")
                    v8 = small.tile([P, 8], f32, tag="v8")
                    nc.vector.max(out=w8, in_=w16)
                    nc.vector.max(out=v8, in_=v16)
                    ge2 = small.tile([P, 2], f32, tag="ge2")
                    nc.vector.tensor_tensor(
                        out=ge2, in0=w8[:, 0:2],
                        in1=v8[:, 5:6].to_broadcast([P, 2]), op=ALU.is_ge)
                    lnw = small.tile([P, 2], f32, tag="lnw")
                    nc.scalar.activation(out=lnw, in_=w8[:, 0:2], func=AF.Ln,
                                         scale=-1.0, bias=1.0)
                    ot = small.tile([P, 4], f32, tag="ot")
                    tmp = small.tile([P, 2], f32, tag="tmp")
                    # bce row-sum = accs[:,0] + ... + accs[:,N_CH-1]
                    nc.vector.tensor_reduce(
                        ot[:, 0:1], accs, axis=mybir.AxisListType.X, op=ALU.add)
                    # selected-negative ln-sum = sum(ge2 * lnw)
                    nc.vector.scalar_tensor_tensor(
                        out=tmp, in0=ge2, scalar=1.0, in1=lnw,
                        op0=ALU.mult, op1=ALU.mult, accum_out=ot[:, 1:2])
                    # mask count = sum(ge2)
                    nc.vector.tensor_reduce(
                        ot[:, 2:3], ge2, axis=mybir.AxisListType.X, op=ALU.add)
                    nc.sync.dma_start(
                        out=stats[rows, 4 * d:4 * d + 3], in_=ot[:, 0:3])

    _split_waits(nc)
    return nc


def _get_nc():
    if "nc" not in _CACHE:
        _CACHE["nc"] = _build()
    return _CACHE["nc"]


def kernel(tk_scores, g_scores, tk_targets, g_targets, confidences):
    nc = _get_nc()
    tk_scores = np.asarray(tk_scores)
    g_scores = np.asarray(g_scores)
    tk_targets = np.asarray(tk_targets)
    g_targets = np.asarray(g_targets)

    in_maps = [
        {
            "tk_s": tk_scores[c * R:(c + 1) * R],
            "tk_t": tk_targets[c * R:(c + 1) * R],
            "g_s": g_scores[c * R:(c + 1) * R],
            "g_t": g_targets[c * R:(c + 1) * R],
        }
        for c in range(N_CORES)
    ]
    res = run_bass_kernel_spmd(nc, in_maps, list(range(N_CORES)))
    stats = np.concatenate(
        [res.results[c]["stats"] for c in range(N_CORES)], axis=0
    ).astype(np.float64)

    conf = np.asarray(confidences, dtype=np.float64)

    def finish(off):
        acc = stats[:, off + 0]      # sum ln(q) per row  (= -row BCE sum)
        negs = stats[:, off + 1]     # sum sel*ln(1-w)    (= -selected loss)
        ms = stats[:, off + 2]
        pos = (conf * -acc).sum() / (B * C)
        neg = (-negs).sum() / (ms.sum() + 1e-8)
        return pos + 0.5 * neg

    tk = finish(0)
    g = finish(4)
    total = 0.6 * tk + 0.4 * g
    return (np.float32(total), np.float32(tk), np.float32(g))
